# revision 18
# baseline (speedup 1.0000x reference)
"""Tsit5 adaptive-RK NeuralODE (B=1024, state 64, MLP 80->128->128->64) on TRN2.

Batch-parallel: 128 trajectories per core x 8 cores. Trajectories live on the
free axis (columns); features on partitions. The MLP first layer is split as
W0 = [W0y | W0u]; since u is constant per trajectory, U0B = W0u@u^T + b0 is a
per-core constant. The RK stage combinations are folded through the first
layer: G = W0y @ W2, so each stage's pre-activation needs only G @ h2_j
accumulations in PSUM (per-term scaled stationaries A_sj*G), plus the constant
column term c_s*(W0y@b2) added via a K=1 outer-product matmul.
"""
import numpy as np
import ml_dtypes
from contextlib import ExitStack

import concourse.tile as tile
from concourse import bacc, mybir
from concourse.bass_utils import run_bass_kernel_spmd

F32 = mybir.dt.float32
BF16 = mybir.dt.bfloat16
U32 = mybir.dt.uint32
ALU = mybir.AluOpType
ACTF = mybir.ActivationFunctionType

B, STATE, CTRL, WIDTH = 1024, 64, 16, 128
NCORES = 8
NB = B // NCORES
MAX_STEPS = 128
UNROLL = 8
USE_BF16 = False

DT0 = 60.0 * (1.0 / 3600.0)
RTOL = ATOL = 1.4e-8
SAFETY, FMIN, FMAX, EXPO = 0.9, 0.1, 10.0, 0.2

A21 = 0.161
A31, A32 = -0.008480655492356989, 0.335480655492357
A41, A42, A43 = 2.8971530571054935, -6.359448489975075, 4.3622954328695815
A51, A52, A53, A54 = (5.325864828439257, -11.748883564062828,
                      7.4955393428898365, -0.09249506636175525)
A61, A62, A63, A64, A65 = (5.86145544294642, -12.92096931784711,
                           8.159367898576159, -0.071584973281401,
                           -0.028269050394068383)
A_ROWS = [
    [A21],
    [A31, A32],
    [A41, A42, A43],
    [A51, A52, A53, A54],
    [A61, A62, A63, A64, A65],
]
B_COEF = [0.09646076681806523, 0.01, 0.4798896504144996, 1.379008574103742,
          -3.290069515436081, 2.324710524099774]
E_COEF = [-0.001780011052226, -0.000816434459657, 0.007880878010262,
          -0.144711007173263, 0.582357165452555, -0.458082105929187,
          -1.0 / 66.0]
N_GT = sum(len(r) for r in A_ROWS)  # 15

LAST_EXEC_NS = None
TRACE = False
_CACHE = {}


def _build(t0: float, t1: float):
    MMDT = BF16 if USE_BF16 else F32
    thr = float(np.float32(t1) - np.float32(1e-12))
    t1f = float(np.float32(t1))

    nc = bacc.Bacc("TRN2", target_bir_lowering=False)

    def din(name, shape, dt=F32):
        return nc.dram_tensor(name, shape, dt, kind="ExternalInput").ap()

    Y0i = din("Y0", [STATE, NB])
    U0Bi = din("U0B", [WIDTH, NB])
    W0YTi = din("W0YT", [STATE, WIDTH], MMDT)
    GTis = [din(f"GT{k}", [WIDTH, WIDTH], MMDT) for k in range(N_GT)]
    W1Ti = din("W1T", [WIDTH, WIDTH], MMDT)
    BETis = [din(f"BET{j}", [WIDTH, WIDTH], MMDT) for j in range(6)]
    E7Ti = din("E7T", [WIDTH, STATE], MMDT)
    CWis = [din(f"CW{s}", [1, WIDTH]) for s in range(5)]
    BERi = din("BER", [1, WIDTH])
    ONES1i = din("ONES1", [1, NB])
    ONES64i = din("ONES64", [STATE, 1], MMDT)
    B1Ci = din("B1C", [WIDTH, 1])
    LN09i = din("LN09", [1, 1])
    OY = nc.dram_tensor("OY", [STATE, NB], F32, kind="ExternalOutput").ap()
    ON = nc.dram_tensor("ON", [1, NB], F32, kind="ExternalOutput").ap()

    with tile.TileContext(nc) as tc, ExitStack() as ctx:
        sb = ctx.enter_context(tc.sbuf_pool(name="sb", bufs=1))
        ps = ctx.enter_context(tc.psum_pool(name="ps", bufs=1))

        def sload(ap_in, shape, dt=F32, name=None):
            tl = sb.tile(shape, dt, name=name)
            nc.sync.dma_start(tl[:], ap_in)
            return tl

        w0yt = sload(W0YTi, [STATE, WIDTH], MMDT, name="w0yt")
        gts = [sload(g, [WIDTH, WIDTH], MMDT, name=f"gt_{k}")
               for k, g in enumerate(GTis)]
        w1t = sload(W1Ti, [WIDTH, WIDTH], MMDT, name="w1t")
        bets = [sload(bti, [WIDTH, WIDTH], MMDT, name=f"bet_{j}")
                for j, bti in enumerate(BETis)]
        e7t = sload(E7Ti, [WIDTH, STATE], MMDT, name="e7t")
        u0b = sload(U0Bi, [WIDTH, NB], name="u0b")
        cwt = [sb.tile([1, WIDTH], F32, name=f"cw_{s}") for s in range(5)]
        for s in range(5):
            nc.sync.dma_start(cwt[s][:], CWis[s])
        ber = sload(BERi, [1, WIDTH], name="ber")
        ones1 = sload(ONES1i, [1, NB], name="ones1")
        ones64 = sload(ONES64i, [STATE, 1], MMDT, name="ones64")
        b1c = sload(B1Ci, [WIDTH, 1], name="b1c")
        ln09 = sload(LN09i, [1, 1], name="ln09")
        y = sload(Y0i, [STATE, NB], name="y")

        t_r = sb.tile([1, NB], F32)
        dt_r = sb.tile([1, NB], F32)
        nst = sb.tile([1, NB], F32)
        nc.vector.memset(t_r[:], float(np.float32(t0)))
        nc.vector.memset(dt_r[:], float(np.float32(DT0)))
        nc.vector.memset(nst[:], 0.0)

        h2 = [sb.tile([WIDTH, NB], MMDT, name=f"h2_{i}") for i in range(7)]
        h0 = sb.tile([WIDTH, NB], MMDT)
        yb = sb.tile([STATE, NB], MMDT)
        y5 = sb.tile([STATE, NB], F32)
        y5b = sb.tile([STATE, NB], MMDT)
        yub = sb.tile([WIDTH, NB], F32)
        d_sb = sb.tile([WIDTH, NB], F32)
        pre = sb.tile([WIDTH, NB], F32)
        tmpw = sb.tile([WIDTH, NB], F32)
        m_u = sb.tile([WIDTH, NB], U32)
        e7sb = sb.tile([STATE, NB], F32)
        ta = sb.tile([STATE, NB], F32)
        tb = sb.tile([STATE, NB], F32)
        err = sb.tile([STATE, NB], F32)
        am = sb.tile([STATE, NB], F32)
        scv = sb.tile([STATE, NB], F32)
        rsc = sb.tile([STATE, NB], F32)
        qv = sb.tile([STATE, NB], F32)
        q2 = sb.tile([STATE, NB], MMDT)

        tr_r = sb.tile([1, NB], F32)
        dtc_r = sb.tile([1, NB], F32)
        en_r = sb.tile([1, NB], F32)
        ln_r = sb.tile([1, NB], F32)
        fac_r = sb.tile([1, NB], F32)
        fc_r = sb.tile([1, NB], F32)
        acc_r = sb.tile([1, NB], F32)
        done_r = sb.tile([1, NB], F32)
        step_r = sb.tile([1, NB], F32)
        nd_r = sb.tile([1, NB], F32)
        nd_u = sb.tile([1, NB], U32)
        sd_r = sb.tile([1, NB], F32)
        df_r = sb.tile([1, NB], F32)

        Dp = ps.tile([WIDTH, NB], F32)
        BDp = ps.tile([WIDTH, NB], F32)
        YUp = ps.tile([WIDTH, NB], F32)
        Tp = ps.tile([WIDTH, NB], F32)
        Hp = ps.tile([WIDTH, NB], F32)
        PEp = ps.tile([WIDTH, NB], F32)
        E7p = ps.tile([STATE, NB], F32)
        en2p = ps.tile([1, NB], F32)

        TT = nc.vector.tensor_tensor
        TS = nc.vector.tensor_scalar
        MM = nc.tensor.matmul
        AV = nc.scalar.activation

        def cast(dst, src):
            nc.scalar.copy(dst[:], src[:])

        def l0_tail(h2out):
            TT(out=pre[:], in0=YUp[:], in1=u0b[:], op=ALU.add)
            AV(h0[:], pre[:], ACTF.Tanh)
            MM(Hp[:], w1t[:], h0[:], start=True, stop=True)
            AV(h2out[:], Hp[:], ACTF.Tanh, bias=b1c[:])

        cast(yb, y)
        MM(YUp[:], w0yt[:], yb[:], start=True, stop=True)
        l0_tail(h2[0])

        def emit_iter():
            # dt_c = min(dt, t1 - t)
            TS(out=tr_r[:], in0=t_r[:], scalar1=-1.0, scalar2=t1f,
               op0=ALU.mult, op1=ALU.add)
            TT(out=dtc_r[:], in0=dt_r[:], in1=tr_r[:], op=ALU.min)
            # D = broadcast dt_c to all 128 partitions (K=1 matmul, exact)
            MM(Dp[:], ones1[:], dtc_r[:], start=True, stop=True)
            cast(d_sb, Dp)
            # BD rows 0:64 = Bsum*b2 x dt_c ; rows 64:128 = Esum*b2 x dt_c
            MM(BDp[:], ber[:], dtc_r[:], start=True, stop=True)
            # YU = W0y @ y ; yub = YU + U0B (includes b0)
            cast(yb, y)
            MM(YUp[:], w0yt[:], yb[:], start=True, stop=True)
            TT(out=yub[:], in0=YUp[:], in1=u0b[:], op=ALU.add)
            k = 0
            for s in range(5):
                na = len(A_ROWS[s])
                for j in range(na):
                    MM(Tp[:], gts[k][:], h2[j][:], start=(j == 0), stop=False)
                    k += 1
                # += c_s*(W0y@b2) broadcast over columns
                MM(Tp[:], cwt[s][:], ones1[:], start=False, stop=True)
                TT(out=tmpw[:], in0=Tp[:], in1=d_sb[:], op=ALU.mult)
                TT(out=pre[:], in0=tmpw[:], in1=yub[:], op=ALU.add)
                AV(h0[:], pre[:], ACTF.Tanh)
                MM(Hp[:], w1t[:], h0[:], start=True, stop=True)
                AV(h2[s + 1][:], Hp[:], ACTF.Tanh, bias=b1c[:])
            # PY (rows 0:64) and errRaw from E1..E6 (rows 64:128)
            for j in range(6):
                MM(PEp[:], bets[j][:], h2[j][:], start=(j == 0), stop=(j == 5))
            # y5 = y + PY*d + BD[0:64]
            TT(out=ta[:], in0=PEp[0:STATE, :], in1=d_sb[0:STATE, :], op=ALU.mult)
            TT(out=tb[:], in0=ta[:], in1=BDp[0:STATE, :], op=ALU.add)
            TT(out=y5[:], in0=tb[:], in1=y[:], op=ALU.add)
            # stage 7 (FSAL candidate)
            cast(y5b, y5)
            MM(YUp[:], w0yt[:], y5b[:], start=True, stop=True)
            l0_tail(h2[6])
            MM(E7p[:], e7t[:], h2[6][:], start=True, stop=True)
            cast(e7sb, E7p)
            # err = (errRaw + E7 term)*d + BD[64:128]
            TT(out=ta[:], in0=PEp[STATE:WIDTH, :], in1=e7sb[:], op=ALU.add)
            TT(out=tb[:], in0=ta[:], in1=d_sb[0:STATE, :], op=ALU.mult)
            TT(out=err[:], in0=tb[:], in1=BDp[STATE:WIDTH, :], op=ALU.add)
            # en = sqrt(mean((err/sc)^2)); sc = ATOL + RTOL*max(|y|,|y5|)
            AV(ta[:], y[:], ACTF.Abs)
            AV(tb[:], y5[:], ACTF.Abs)
            TT(out=am[:], in0=ta[:], in1=tb[:], op=ALU.max)
            TS(out=scv[:], in0=am[:], scalar1=RTOL, scalar2=ATOL,
               op0=ALU.mult, op1=ALU.add)
            nc.vector.reciprocal(rsc[:], scv[:])
            TT(out=qv[:], in0=err[:], in1=rsc[:], op=ALU.mult)
            TT(out=q2[:], in0=qv[:], in1=qv[:], op=ALU.mult)
            MM(en2p[:], ones64[:], q2[:], start=True, stop=True)
            AV(en_r[:], en2p[:], ACTF.Sqrt, scale=1.0 / STATE)
            # fac = clip(0.9 * en^-0.2, 0.1, 10)
            AV(ln_r[:], en_r[:], ACTF.Ln)
            AV(fac_r[:], ln_r[:], ACTF.Exp, bias=ln09[:], scale=-EXPO)
            TS(out=fc_r[:], in0=fac_r[:], scalar1=FMIN, scalar2=FMAX,
               op0=ALU.max, op1=ALU.min)
            # masks
            TS(out=acc_r[:], in0=en_r[:], scalar1=1.0, scalar2=0.0,
               op0=ALU.is_le, op1=ALU.add)
            TS(out=done_r[:], in0=t_r[:], scalar1=thr, scalar2=0.0,
               op0=ALU.is_ge, op1=ALU.add)
            TS(out=nd_r[:], in0=t_r[:], scalar1=thr, scalar2=0.0,
               op0=ALU.is_lt, op1=ALU.add)
            TS(out=nd_u[:], in0=t_r[:], scalar1=thr, scalar2=0.0,
               op0=ALU.is_lt, op1=ALU.add)
            TT(out=step_r[:], in0=acc_r[:], in1=done_r[:], op=ALU.is_gt)
            # t += step*dt_c
            TT(out=sd_r[:], in0=step_r[:], in1=dtc_r[:], op=ALU.mult)
            TT(out=t_r[:], in0=t_r[:], in1=sd_r[:], op=ALU.add)
            # dt = where(done, dt, dt_c*fac)
            TT(out=df_r[:], in0=dtc_r[:], in1=fc_r[:], op=ALU.mult)
            nc.vector.copy_predicated(dt_r[:], nd_u[:], df_r[:])
            # nsteps += ~done
            TT(out=nst[:], in0=nst[:], in1=nd_r[:], op=ALU.add)
            # broadcast step mask, predicated state update (y, FSAL h2_1)
            MM(Tp[:], ones1[:], step_r[:], start=True, stop=True)
            TS(out=m_u[:], in0=Tp[:], scalar1=0.5, scalar2=0.0,
               op0=ALU.is_ge, op1=ALU.add)
            nc.vector.copy_predicated(y[:], m_u[0:STATE, :], y5[:])
            nc.vector.copy_predicated(h2[0][:], m_u[:], h2[6][:])

        with tc.For_i(0, MAX_STEPS // UNROLL) as _i:
            for _ in range(UNROLL):
                emit_iter()

        nc.sync.dma_start(OY, y[:])
        nc.sync.dma_start(ON, nst[:])

    nc.compile()
    return nc


def _precompute(inputs):
    f32 = np.float32
    x0 = np.asarray(inputs["x0"], f32)
    u = np.asarray(inputs["u"], f32)
    W0 = np.asarray(inputs["W0"], f32)
    b0 = np.asarray(inputs["b0"], f32)
    W1 = np.asarray(inputs["W1"], f32)
    b1 = np.asarray(inputs["b1"], f32)
    W2 = np.asarray(inputs["W2"], f32)
    b2 = np.asarray(inputs["b2"], f32)

    W0y = W0[:, :STATE]
    W0u = W0[:, STATE:]
    G = (W0y @ W2).astype(f32)
    w0yb2 = (W0y @ b2).astype(f32)
    CS = [f32(sum(r)) for r in A_ROWS]
    Bsum = f32(sum(B_COEF))
    Esum = f32(sum(E_COEF))

    if USE_BF16:
        def mmcast(a):
            return np.ascontiguousarray(a.astype(f32)).astype(ml_dtypes.bfloat16)
    else:
        def mmcast(a):
            return np.ascontiguousarray(a, f32)

    consts = {}
    k = 0
    for rowa in A_ROWS:
        for a in rowa:
            consts[f"GT{k}"] = mmcast((f32(a) * G).T)
            k += 1
    consts["W0YT"] = mmcast(W0y.T)
    consts["W1T"] = mmcast(W1.T)
    for j in range(6):
        stack = np.vstack([f32(B_COEF[j]) * W2, f32(E_COEF[j]) * W2]).astype(f32)
        consts[f"BET{j}"] = mmcast(stack.T)
    consts["E7T"] = mmcast((f32(E_COEF[6]) * W2).T)
    for s, cs in enumerate(CS):
        consts[f"CW{s}"] = (cs * w0yb2).astype(f32)[None, :]
    consts["BER"] = np.concatenate([Bsum * b2, Esum * b2]).astype(f32)[None, :]
    consts["ONES1"] = np.ones((1, NB), f32)
    consts["ONES64"] = mmcast(np.ones((STATE, 1), f32))
    consts["B1C"] = np.ascontiguousarray(b1.reshape(WIDTH, 1))
    consts["LN09"] = np.array([[np.log(f32(SAFETY))]], f32)

    in_maps = []
    for c in range(NCORES):
        sl = slice(c * NB, (c + 1) * NB)
        m = dict(consts)
        m["Y0"] = np.ascontiguousarray(x0[sl].T)
        m["U0B"] = (W0u @ u[sl].T + b0[:, None]).astype(f32)
        in_maps.append(m)
    return in_maps


def kernel(**inputs):
    global LAST_EXEC_NS
    t0 = float(np.float32(inputs["t0"]))
    t1 = float(np.float32(inputs["t1"]))
    key = (t0, t1, USE_BF16, UNROLL)
    if key not in _CACHE:
        _CACHE[key] = _build(t0, t1)
    nc = _CACHE[key]
    in_maps = _precompute(inputs)
    res = run_bass_kernel_spmd(nc, in_maps, list(range(NCORES)), trace=TRACE)
    LAST_EXEC_NS = res.exec_time_ns
    y_full = np.empty((B, STATE), np.float32)
    ns_full = np.empty((B,), np.int32)
    for c in range(NCORES):
        r = res.results[c]
        y_full[c * NB:(c + 1) * NB] = r["OY"].T
        ns_full[c * NB:(c + 1) * NB] = np.rint(r["ON"][0]).astype(np.int32)
    return y_full, ns_full
